# revision 3
# baseline (speedup 1.0000x reference)
"""Multi-head attention (B=4, S=2048, D=768, H=12) on 8 NeuronCores.

Sharding: core c handles batch b = c//2 and head-group g = c%2 (6 heads,
384 of the 768 QKV output features; Wo row-slice).  Each core computes a
partial output-projection outT [768, 2048]; the host sums the two
head-group partials per batch, transposes, and adds bo.

Device-side layout trick: all projections are computed *transposed*
(features on partitions) by passing x^T (host-prepared) as the matmul
moving operand, so attention scores can be formed directly as
S^T = K Q^T with the key dim on partitions.  Softmax runs without max
subtraction (scores are ~N(0,1)); the denominator falls out of the
P@V matmul for free via a ones-column appended to V.  Normalization is
applied per q-row right before the PE transpose that builds attnT.
"""

import os
import sys
from contextlib import ExitStack

import numpy as np

for _p in ("/opt/trn_rl_repo", "/root/.axon_site/_ro/trn_rl_repo"):
    if os.path.isdir(_p) and _p not in sys.path:
        sys.path.insert(0, _p)

import ml_dtypes  # noqa: E402

import concourse.bass as bass  # noqa: E402
import concourse.mybir as mybir  # noqa: E402
import concourse.tile as tile  # noqa: E402
from concourse import bacc  # noqa: E402
from concourse import bass_utils  # noqa: E402
from concourse.masks import make_identity  # noqa: E402

F32 = mybir.dt.float32
F32R = mybir.dt.float32r
BF16 = mybir.dt.bfloat16
P = 128

B, S, D, H = 4, 2048, 768, 12
HS = D // H           # 64
N_CORES = 8
GROUPS = 2            # head groups (cores per batch)
HPC = H // GROUPS     # 6 heads per core
DOUT = HPC * HS       # 384


def build(S=S, D=D, HPC=HPC, hs=HS, QBLK=1024):
    """Emit the per-core Bass program (SPMD: same program on all cores)."""
    nc = bacc.Bacc("TRN2", debug=False, num_devices=N_CORES)

    DOUT = HPC * hs
    NDIN = D // P            # x-feature (contraction) chunks
    NDT = DOUT // P          # q/k feature tiles
    NKT = S // P             # key tiles
    QBLK = min(QBLK, S)
    NQB = S // QBLK          # q blocks processed at a time
    NQT = QBLK // P          # q tiles per block
    SC = 512
    NSC = S // SC
    scale = float(hs) ** -0.5

    xT_d = nc.dram_tensor("xT", [D, S], F32R, kind="ExternalInput")
    wq_d = nc.dram_tensor("wq", [D, DOUT], F32R, kind="ExternalInput")
    wk_d = nc.dram_tensor("wk", [D, DOUT], F32R, kind="ExternalInput")
    wv_d = nc.dram_tensor("wv", [D, DOUT], F32R, kind="ExternalInput")
    wo_d = nc.dram_tensor("wo", [DOUT, D], BF16, kind="ExternalInput")
    bq_d = nc.dram_tensor("bq", [DOUT], F32, kind="ExternalInput")
    bk_d = nc.dram_tensor("bk", [DOUT], F32, kind="ExternalInput")
    bv_d = nc.dram_tensor("bv", [DOUT], F32R, kind="ExternalInput")
    out_d = nc.dram_tensor("outT", [D, S], F32, kind="ExternalOutput")

    with ExitStack() as ctx:
        tc = ctx.enter_context(tile.TileContext(nc))
        const = ctx.enter_context(tc.tile_pool(name="const", bufs=1))
        big = ctx.enter_context(tc.tile_pool(name="big", bufs=1))
        expp = ctx.enter_context(tc.tile_pool(name="expp", bufs=NKT))
        evp = ctx.enter_context(tc.tile_pool(name="evp", bufs=3))
        outp = ctx.enter_context(tc.tile_pool(name="outp", bufs=2))
        ps_pr = ctx.enter_context(tc.tile_pool(name="ps_pr", bufs=1, space="PSUM"))
        ps_s = ctx.enter_context(tc.tile_pool(name="ps_s", bufs=2, space="PSUM"))
        ps_o = ctx.enter_context(tc.tile_pool(name="ps_o", bufs=2, space="PSUM"))
        ps_t = ctx.enter_context(tc.tile_pool(name="ps_t", bufs=1, space="PSUM"))

        ident = const.tile([P, P], BF16)
        make_identity(nc, ident)
        ones_f = const.tile([1, P], F32)
        nc.gpsimd.memset(ones_f, 1.0)
        ones = const.tile([1, P], F32R)
        nc.vector.tensor_copy(ones, ones_f)

        xT = big.tile([P, NDIN, S], F32R)
        nc.sync.dma_start(xT, xT_d.ap().rearrange("(c p) s -> p c s", p=P))
        wq = big.tile([P, NDIN, DOUT], F32R)
        nc.sync.dma_start(wq, wq_d.ap().rearrange("(c p) n -> p c n", p=P))
        wk = big.tile([P, NDIN, DOUT], F32R)
        nc.sync.dma_start(wk, wk_d.ap().rearrange("(c p) n -> p c n", p=P))
        wv = big.tile([P, NDIN, DOUT], F32R)
        nc.sync.dma_start(wv, wv_d.ap().rearrange("(c p) n -> p c n", p=P))
        wo = big.tile([P, NDT, D], BF16)
        nc.sync.dma_start(wo, wo_d.ap().rearrange("(c p) n -> p c n", p=P))
        bqs = const.tile([P, NDT], F32)
        nc.sync.dma_start(bqs, bq_d.ap().rearrange("(c p) -> p c", p=P))
        bks = const.tile([P, NDT], F32)
        nc.sync.dma_start(bks, bk_d.ap().rearrange("(c p) -> p c", p=P))
        bvs = const.tile([1, DOUT], F32R)
        nc.sync.dma_start(bvs, bv_d.ap()[None, :])

        # ---- K^T / Q^T projections: [feat, seq], feat on partitions ----
        KT = big.tile([P, NDT, S], F32R)
        QT = big.tile([P, NDT, S], F32R)
        for W, BS, OUT in ((wk, bks, KT), (wq, bqs, QT)):
            for t in range(NDT):
                for sj in range(NSC):
                    ps = ps_pr.tile([P, SC], F32, tag="pp", name="ps_pr")
                    for c in range(NDIN):
                        nc.tensor.matmul(
                            ps,
                            W[:, c, t * P:(t + 1) * P],
                            xT[:, c, sj * SC:(sj + 1) * SC],
                            start=(c == 0),
                            stop=(c == NDIN - 1),
                        )
                    nc.vector.tensor_scalar_add(
                        OUT[:, t, sj * SC:(sj + 1) * SC], ps, BS[:, t:t + 1]
                    )

        # ---- V in [seq, feat] layout, with ones column for softmax denom ----
        Vt = big.tile([P, NKT, HPC, hs + 1], BF16)
        nc.gpsimd.memset(Vt[:, :, :, hs:hs + 1], 1.0)
        for st in range(NKT):
            ps = ps_pr.tile([P, DOUT], F32, tag="pp", name="ps_pr")
            for c in range(NDIN):
                nc.tensor.matmul(
                    ps,
                    xT[:, c, st * P:(st + 1) * P],
                    wv[:, c, :],
                    start=(c == 0),
                    stop=False,
                )
            nc.tensor.matmul(ps, ones, bvs, start=False, stop=True)
            nc.vector.tensor_copy(
                Vt[:, st, :, 0:hs], ps.rearrange("p (h d) -> p h d", d=hs)
            )

        # ---- attention ----
        attnT = big.tile([P, NDT, S], BF16)
        for h in range(HPC):
            ch, off = h // 2, (h % 2) * 64
            for qb in range(NQB):
                q0 = qb * QBLK
                exps = []
                for kt in range(NKT):
                    ps_sc = ps_s.tile([P, QBLK], F32, tag="ps_s", name="ps_s")
                    for qc in range(QBLK // SC):
                        nc.tensor.matmul(
                            ps_sc[:, qc * SC:(qc + 1) * SC],
                            KT[off:off + 64, ch, kt * P:(kt + 1) * P],
                            QT[off:off + 64, ch, q0 + qc * SC:q0 + (qc + 1) * SC],
                            start=True,
                            stop=True,
                        )
                    ex = expp.tile([P, QBLK], BF16, tag="expS", name="expS")
                    nc.scalar.activation(
                        ex, ps_sc, mybir.ActivationFunctionType.Exp, scale=scale
                    )
                    exps.append(ex)
                for qtp in range(NQT // 2):
                    pos = [
                        ps_o.tile([P, hs + 1], F32, tag="ps_o", name="ps_o")
                        for _ in range(2)
                    ]
                    for kt in range(NKT):
                        for j in range(2):
                            qt = qtp * 2 + j
                            nc.tensor.matmul(
                                pos[j],
                                exps[kt][:, qt * P:(qt + 1) * P],
                                Vt[:, kt, h, :],
                                start=(kt == 0),
                                stop=(kt == NKT - 1),
                            )
                    for j in range(2):
                        qt = qtp * 2 + j
                        rcp = evp.tile([P, 1], F32, tag="rcp", name="rcp")
                        nc.vector.reciprocal(rcp, pos[j][:, hs:hs + 1])
                        onorm = evp.tile([P, hs], BF16, tag="onorm", name="onorm")
                        nc.vector.tensor_scalar_mul(onorm, pos[j][:, 0:hs], rcp)
                        pt = ps_t.tile([P, P], BF16, tag="pt", name="pt")
                        nc.tensor.transpose(pt[off:off + 64, :], onorm, ident)
                        nc.vector.tensor_copy(
                            attnT[off:off + 64, ch, q0 + qt * P:q0 + (qt + 1) * P],
                            pt[off:off + 64, :],
                        )

        # ---- output projection (partial over this core's 384 dims) ----
        out_r = out_d.ap().rearrange("(t p) s -> t p s", p=P)
        for t in range(D // P):
            for sj in range(NSC):
                ps = ps_pr.tile([P, SC], F32, tag="pp", name="ps_pr")
                for c in range(NDT):
                    nc.tensor.matmul(
                        ps,
                        wo[:, c, t * P:(t + 1) * P],
                        attnT[:, c, sj * SC:(sj + 1) * SC],
                        start=(c == 0),
                        stop=(c == NDT - 1),
                    )
                ob = outp.tile([P, SC], F32, tag="ob", name="ob")
                nc.scalar.copy(ob, ps)
                nc.sync.dma_start(out_r[t, :, sj * SC:(sj + 1) * SC], ob)

    nc.compile()
    return nc


def make_in_maps(x, Wq, bq, Wk, bk, Wv, bv, Wo, bo):
    """Split full inputs into the 8 per-core input dicts."""
    in_maps = []
    for c in range(N_CORES):
        b, g = c // 2, c % 2
        sl = slice(g * DOUT, (g + 1) * DOUT)
        in_maps.append({
            "xT": np.ascontiguousarray(np.asarray(x[b], np.float32).T),
            "wq": np.ascontiguousarray(np.asarray(Wq, np.float32)[:, sl]),
            "wk": np.ascontiguousarray(np.asarray(Wk, np.float32)[:, sl]),
            "wv": np.ascontiguousarray(np.asarray(Wv, np.float32)[:, sl]),
            "wo": np.ascontiguousarray(
                np.asarray(Wo, np.float32)[sl, :].astype(ml_dtypes.bfloat16)
            ),
            "bq": np.ascontiguousarray(np.asarray(bq, np.float32)[sl]),
            "bk": np.ascontiguousarray(np.asarray(bk, np.float32)[sl]),
            "bv": np.ascontiguousarray(np.asarray(bv, np.float32)[sl]),
        })
    return in_maps


_NC_CACHE = {}


def _get_nc():
    if "nc" not in _NC_CACHE:
        _NC_CACHE["nc"] = build()
    return _NC_CACHE["nc"]


def run(inputs, trace=False, **kwargs):
    """Run on 8 cores; returns (full_output, BassKernelResults)."""
    nc = _get_nc()
    in_maps = make_in_maps(**inputs)
    res = bass_utils.run_bass_kernel_spmd(
        nc, in_maps, core_ids=list(range(N_CORES)), trace=trace, **kwargs
    )
    bo = np.asarray(inputs["bo"], np.float32)
    out = np.empty((B, S, D), np.float32)
    for b in range(B):
        acc = res.results[2 * b]["outT"] + res.results[2 * b + 1]["outT"]
        out[b] = acc.T + bo
    return out, res


def kernel(**inputs):
    out, _ = run(inputs, trace=False)
    return out
